# revision 51
# baseline (speedup 1.0000x reference)
"""Trainium2 Bass kernel for nn_MultiHeadAttention (B=2, S=2048, D=1024, H=16).

Sharding (8 cores): data-parallel over batch (2) x tensor-parallel over
head groups (4 groups of 4 heads).  Core c handles batch c//4, heads
4*(c%4) .. 4*(c%4)+3 plus its slice of the output projection; the host
sums the 4 partial output projections per batch and adds bo.

Design notes (~253us baseline -> ~218-220us):
  * scores matmuls run ROW-TILED (K=64 head dim -> tile_size (64,128)):
    the two heads of a pair live on SBUF partitions 0-63 / 64-127, so
    their score matmuls land on PE row-tiles T0/T8 and stream
    CONCURRENTLY (measured ~118ns/MM vs 215 serial).  They are emitted
    under tc.high_priority so the tile scheduler keeps the pair
    adjacent in the PE stream (it otherwise splits them).
  * one exp() activation per (pair, key tile) covers both heads
    ([128,1024] PSUM -> bf16 SBUF, ~1.1us/call, 128 calls ~= 134us
    busy); exp is the #2 engine after the PE (~185us streaming work).
  * PSUM: 2x sc [128,1024] (4 banks) + 2x ctx accum (2) + 2x proj
    accum (2) = 8 banks exactly; ctx accumulates v'@ex over 16 key
    tiles with a fused ones-column giving the softmax denominators.
  * q/k/v/out projections are deadline-scheduled filler granules popped
    between attention steps; ctx matmuls drain lagged behind exp so the
    PE never waits on the activation right before a scores pair.
  * inputs are HOST-PACKED per partition (contiguous DMA rows; the
    naive [p,k,c] gather measured only ~85GB/s) and stream in
    need-order chains with the six criticals split k-halves across
    sync+gpsimd (a single queue caps at ~110-135GB/s; bulk stays on
    sync only -- bulk on gpsimd blocks its queue and delays the norm
    broadcasts, measured +7us).
  * outproj j>=2 is reserved for the tail with accumulators spread
    over the freed sc banks and copies split across scalar/vector;
    output is written bf16 (host sums the 4 partials in fp32).
"""

import sys

for _p in ("/opt/trn_rl_repo",):
    if _p not in sys.path:
        sys.path.insert(0, _p)

from contextlib import ExitStack

import ml_dtypes
import numpy as np

import concourse.bass as bass
import concourse.tile as tile
from concourse import bacc, mybir
from concourse.bass_utils import run_bass_kernel_spmd

B, S, D, H = 2, 2048, 1024, 16
HD = D // H            # 64 head dim
NG = 4                 # head groups (cores per batch)
NHC = H // NG          # 4 heads per core
FS = NHC * HD          # 256 features per core
P = 128
DK = D // P            # 8 contraction tiles for projections
SK = S // P            # 16 key tiles
NQ = S // 512          # 4 query chunks
VW = HD + 1            # v feats + ones column

f32 = mybir.dt.float32
bf16 = mybir.dt.bfloat16
EXP = mybir.ActivationFunctionType.Exp
EXBUFS = 12            # ex tile ring (must exceed max ctx-drain backlog)
SCALE = 1.0 / (HD ** 0.5)


def _emit(ctx: ExitStack, tc, nc, io):
    QT, KT, VT, WqT, WkT, WvT, WoT, bq, bk, bv, OUTP = io

    # ---- pools (PSUM pools first => bank-aligned slots) ----
    sc_ps = ctx.enter_context(tc.tile_pool(name="sc_ps", bufs=2, space="PSUM"))
    ctx_ps = ctx.enter_context(tc.tile_pool(name="ctx_ps", bufs=2, space="PSUM"))
    acc_ps = ctx.enter_context(tc.tile_pool(name="acc_ps", bufs=2, space="PSUM"))
    wp = ctx.enter_context(tc.tile_pool(name="wp", bufs=1))
    per = ctx.enter_context(tc.tile_pool(name="per", bufs=1))
    exq = ctx.enter_context(tc.tile_pool(name="exq", bufs=EXBUFS))
    nrm = ctx.enter_context(tc.tile_pool(name="nrm", bufs=2))
    cnp = ctx.enter_context(tc.tile_pool(name="cnp", bufs=2))
    outp = ctx.enter_context(tc.tile_pool(name="outp", bufs=3))

    # ---- persistent SBUF ----
    wk_all = wp.tile([P, DK * FS], bf16, tag="wk")   # [p, (k, fs)]
    wq_all = wp.tile([P, DK * FS], bf16, tag="wq")
    wv_all = wp.tile([P, DK * FS], bf16, tag="wv")
    wo_all = wp.tile([P, 2 * D], bf16, tag="wo")     # [p, (f, d)]
    bqt = wp.tile([P, 2], f32, tag="bqt")            # [p, f]
    bkt = wp.tile([P, 2], f32, tag="bkt")
    bvt = wp.tile([P, FS], f32, tag="bvt")
    KTi = wp.tile([P, DK * S], bf16, tag="KTi")      # [p, (k, c)]
    QTi = wp.tile([P, DK * S], bf16, tag="QTi")
    VTi = wp.tile([P, DK * S], bf16, tag="VTi")
    kT = [per.tile([P, S], bf16, tag=f"kT{f}", name=f"kT{f}") for f in range(2)]
    qT = [per.tile([P, S], bf16, tag=f"qT{f}", name=f"qT{f}") for f in range(2)]
    vsb = [per.tile([P, NHC * VW], bf16, tag=f"v{t}", name=f"v{t}")
           for t in range(SK)]

    KTi3 = KTi.rearrange("p (k c) -> p k c", c=S)
    QTi3 = QTi.rearrange("p (k c) -> p k c", c=S)
    VTi3 = VTi.rearrange("p (k c) -> p k c", c=S)

    # ---- exp table load (cold matmul warm-up is counterproductive:
    # cold MMs run at ~50% duty and never trip HAM; dense projection
    # work warms the PE in ~3.4us on its own) ----
    warm_sb = wp.tile([P, 16], bf16, tag="warm")
    nc.vector.memset(warm_sb[:], 0.0)
    warm_ex = wp.tile([P, 16], bf16, tag="warmex")
    nc.scalar.activation(warm_ex[:], warm_sb[:], EXP, scale=0.125)

    # ---- input DMAs, priority order ----
    def qslice(dram3, q):
        return dram3[:, :, q * 512:(q + 1) * 512]


    # Inputs are HOST-PACKED so every DMA is contiguous per partition
    # (the [p, k, c] gather pattern measured only ~85 GB/s; contiguous
    # rows run at full HBM rate).  Seq tensors are packed quarter-major:
    # dram[p, q, k, c] = XT[k*128+p, q*512+c].
    # K criticals first and ALONE at full bandwidth (sync chain); the Q
    # chain (gpsimd) is gated behind KTq0 by a dummy copy dep; V + bulk
    # chain behind K on sync; biases on scalar (tiny).
    def qsrc(dram, q):
        return dram[:, q * 4096:(q + 1) * 4096].rearrange(
            "p (k c) -> p k c", c=512)

    nc.scalar.dma_start(bkt[:], bk[:, :])
    nc.scalar.dma_start(bqt[:], bq[:, :])
    nc.scalar.dma_start(bvt[:], bv.to_broadcast((P, FS)))
    # criticals split k-halves across sync+gpsimd (a single queue moves
    # only ~110-135 GB/s total); each queue's chain is in need-order so
    # in-queue descriptor sequencing keeps the priority
    def crit(dst3, src3):
        nc.sync.dma_start(dst3[:, 0:4], src3[:, 0:4])
        nc.gpsimd.dma_start(dst3[:, 4:8], src3[:, 4:8])

    def w3(dst, src):
        return (dst.rearrange("p (k c) -> p k c", c=FS),
                src.rearrange("p (k c) -> p k c", c=FS))

    crit(*w3(wk_all, WkT))
    crit(qslice(KTi3, 0), qsrc(KT, 0))
    crit(*w3(wq_all, WqT))
    crit(qslice(QTi3, 0), qsrc(QT, 0))
    crit(*w3(wv_all, WvT))
    crit(qslice(VTi3, 0), qsrc(VT, 0))
    for q in (1, 2, 3):
        nc.sync.dma_start(qslice(KTi3, q), qsrc(KT, q))
        nc.sync.dma_start(qslice(VTi3, q), qsrc(VT, q))
    nc.sync.dma_start(wo_all[:], WoT[:, :])
    for q in (1, 2, 3):
        nc.sync.dma_start(qslice(QTi3, q), qsrc(QT, q))

    # ================= filler granules =================
    # Each projection quarter is a 2-granule sequence [open, close] over
    # one acc_ps accumulator; at most 2 sequences may be open at a time.
    kq_state = {}

    def kq_granule(dst, w_all, b_t, src3, f, q, part, label=""):
        def g():
            key = (label, f, q)
            if part == 0:
                kq_state[key] = acc_ps.tile([P, 512], f32, tag="acc",
                                            name="pacc")
            ps = kq_state.pop(key) if part == 1 else kq_state[key]
            for k in range(4 * part, 4 * part + 4):
                nc.tensor.matmul(
                    ps[:], w_all[:, k * FS + f * P: k * FS + (f + 1) * P],
                    src3[:, k, q * 512:(q + 1) * 512],
                    start=(k == 0), stop=(k == DK - 1))
            if part == 1:
                nc.vector.tensor_scalar_add(
                    dst[f][:, q * 512:(q + 1) * 512], ps[:], b_t[:, f:f + 1])
        return g

    v_state = {}

    def v_granule(t, part):
        def g():
            if part == 0:
                v_state[t] = acc_ps.tile([P, FS], f32, tag="acc", name="vacc")
            ps = v_state.pop(t) if part == 1 else v_state[t]
            for k in range(4 * part, 4 * part + 4):
                nc.tensor.matmul(
                    ps[:], VTi3[:, k, t * P:(t + 1) * P],
                    wv_all[:, k * FS:(k + 1) * FS],
                    start=(k == 0), stop=(k == DK - 1))
            if part == 1:
                v3 = vsb[t].rearrange("p (h w) -> p h w", w=VW)
                nc.vector.tensor_add(
                    v3[:, :, 0:HD],
                    ps.rearrange("p (h w) -> p h w", w=HD),
                    bvt.rearrange("p (h w) -> p h w", w=HD))
                nc.vector.memset(v3[:, :, HD:VW], 1.0)
        return g

    def o_granule(j, mt, oc, cn, ob_box, eng=None, use_sc=False):
        def g():
            if oc == 0:
                ob_box.append(outp.tile([P, 1024], bf16, tag="ob", name="ob"))
            ob = ob_box[-1]
            if use_sc:
                # tail only: scores are done, reuse the sc PSUM banks so
                # the outproj accumulators rotate over 4 banks
                big = sc_ps.tile([P, 1024], f32, tag="sc", name="oacc2")
                ps = big[:, 0:512]
            else:
                ps = acc_ps.tile([P, 512], f32, tag="acc", name="oacc")
            for f in range(2):
                nc.tensor.matmul(
                    ps[:], cn[f][:, mt * P:(mt + 1) * P],
                    wo_all[:, f * D + oc * 512: f * D + (oc + 1) * 512],
                    start=(f == 0), stop=(f == 1))
            dst = ob[:, oc * 512:(oc + 1) * 512]
            if eng == "scalar":
                nc.scalar.copy(dst, ps[:])
            elif eng == "gpsimd":
                nc.gpsimd.tensor_copy(dst, ps[:])
            else:
                nc.vector.tensor_copy(dst, ps[:])
            if oc == 1:
                nc.gpsimd.dma_start(
                    OUTP[j * 512 + mt * P: j * 512 + (mt + 1) * P, :], ob[:])
        return g

    # ---- filler bookkeeping ----
    # fillers[(sid, part)] = [deadline, earliest, fn].  Sequences of kind
    # k/q/v share one acc_ps accumulator across their two granules; at
    # most ONE such sequence may be open (part 0 popped, part 1 not).
    fillers = {}
    state = {"open": None}
    vsb_emitted = set()

    def add_seq(sid, earliest, deadline, fns):
        for part, fn in enumerate(fns):
            fillers[(sid, part)] = [deadline, earliest, fn]

    def _pop(sid, part):
        ent = fillers.pop((sid, part), None)
        if ent is None:
            return False
        ent[2]()
        if sid[0] in ("k", "q", "v"):
            state["open"] = sid if part == 0 else None
        if sid[0] == "v" and part == 1:
            vsb_emitted.add(sid[1])
        return True

    def close_open():
        if state["open"] is not None:
            _pop(state["open"], 1)

    def pop_seq_now(sid):
        if state["open"] is not None and state["open"] != sid:
            close_open()
        _pop(sid, 0)
        _pop(sid, 1)

    def scheduler_pop(astep, budget):
        # pop all past-deadline granules (free) + up to `budget` extras
        spent = 0
        while True:
            if state["open"] is not None:
                sid = state["open"]
                ent = fillers.get((sid, 1))
                due = ent is not None and ent[0] <= astep
                if not due and spent >= budget:
                    return
                _pop(sid, 1)
                if not due:
                    spent += 1
                continue
            best = None
            for (sid, part), ent in fillers.items():
                if part == 1 and (sid, 0) in fillers:
                    continue
                due = ent[0] <= astep
                if not due and ent[1] > astep:
                    continue
                key = (0 if due else 1, ent[0], ent[1], str(sid))
                if best is None or key < best[0]:
                    best = (key, sid, part, due)
            if best is None:
                return
            if not best[3] and spent >= budget:
                return
            _pop(best[1], best[2])
            if not best[3]:
                spent += 1

    # register filler sequences
    # kproj quarters: f is the pair index; scores (j=0,p,kt) need q=kt//4
    for f in range(2):
        for q in range(4):
            first_use = f * SK + 4 * q
            add_seq(("k", f, q), max(0, first_use - 8), first_use - 3,
                    [kq_granule(kT, wk_all, bkt, KTi3, f, q, p2, "k")
                     for p2 in range(2)])
    # qproj: qT[f] quarter j needed at astep (j*2+f)*SK
    for f in range(2):
        for j in range(NQ):
            first_use = (j * 2 + f) * SK
            add_seq(("q", f, j), max(0, first_use - 10), first_use - 4,
                    [kq_granule(qT, wq_all, bqt, QTi3, f, j, p2, "q")
                     for p2 in range(2)])
    # vproj: vsb[t] needed by ctx drain of (j=0, p=0, kt=t)
    for t in range(SK):
        add_seq(("v", t), max(0, t - 4), t,
                [v_granule(t, p2) for p2 in range(2)])

    # ================= attention =================
    pending = []           # (pair_serial, kt, ex, emit_astep)
    pair_cp = {}           # pair_serial -> [cp_even, cp_odd]
    pair_drained = {}
    ndrained = 0
    cn_byj = {}

    def drain_one():
        nonlocal ndrained
        ps_serial, kt, ex, _ = pending.pop(0)
        j, p = divmod(ps_serial, 2)
        if kt not in vsb_emitted:
            pop_seq_now(("v", kt))
        if ps_serial not in pair_cp:
            pair_cp[ps_serial] = [
                ctx_ps.tile([VW, 512], f32, tag="ctx", name=f"cp{ps_serial}h{i}")
                for i in range(2)]
            pair_drained[ps_serial] = 0
        cps = pair_cp[ps_serial]
        nd = pair_drained[ps_serial]
        for i in range(2):
            h = 2 * p + i
            nc.tensor.matmul(
                cps[i][:], vsb[kt][:, h * VW:(h + 1) * VW],
                ex[:, i * 512:(i + 1) * 512],
                start=(nd == 0), stop=(nd == SK - 1))
        pair_drained[ps_serial] = nd + 1
        ndrained += 1
        if nd == SK - 1:
            finish_pair(ps_serial)

    def finish_pair(ps_serial):
        j, p = divmod(ps_serial, 2)
        last = ps_serial == 2 * NQ - 1
        if j not in cn_byj:
            cn_byj[j] = [cnp.tile([P, 512], bf16, tag=f"cn{f}", name=f"cn{f}")
                         for f in range(2)]
        cn = cn_byj[j]
        cps = pair_cp.pop(ps_serial)
        for i in range(2):
            h = 2 * p + i
            if last:
                # tail: no PSUM pressure; skip staging, shortest chain
                src, srcsum = cps[i], cps[i][HD:HD + 1, :]
            else:
                cu = nrm.tile([VW, 512], f32, tag="cu", name="cu")
                with tc.high_priority(offset=5 * 10 ** 5):
                    nc.vector.tensor_copy(cu[:], cps[i][:])  # frees PSUM
                src, srcsum = cu, cu[HD:HD + 1, :]
            sm = nrm.tile([1, 512], f32, tag="sm", name="sm")
            nc.vector.tensor_copy(sm[:], srcsum)
            r1 = nrm.tile([1, 512], f32, tag="r1", name="r1")
            t1 = nrm.tile([1, 512], f32, tag="t1", name="t1")
            nc.vector.reciprocal_approx_accurate(r1[:], sm[:], t1[:])
            rb = nrm.tile([HD, 512], f32, tag="rb", name="rb")
            nc.gpsimd.partition_broadcast(rb[:], r1[:])
            nc.vector.tensor_mul(cn[p][(i) * HD:(i + 1) * HD, :],
                                 src[0:HD, :], rb[:])
        if p == 1:
            # both pairs of qchunk j normed -> register outproj fillers.
            # j>=2 outproj is reserved for the tail (keeps the PE warm
            # while the final norm chain runs).
            here = ps_serial * SK + SK - 1
            tail_j = j >= NQ - 2
            dl = 10 ** 9 if tail_j else (j + 1) * 2 * SK + 24
            early = 10 ** 9 if tail_j else here + 2
            for mt in range(4):
                ob_box = []
                eng = ("scalar", "vector") if tail_j else (None, None)
                add_seq(("o", j, mt), early, dl,
                        [o_granule(j, mt, oc, cn, ob_box, eng=eng[oc],
                                   use_sc=tail_j and (mt * 2 + oc) % 2 == 0)
                         for oc in (0, 1)])

    for j in range(NQ):
        for p in range(2):
            if j == 0:
                pop_seq_now(("k", p, 0))   # kproj first: K data lands first
            pop_seq_now(("q", p, j))
            for kt in range(SK):
                astep = (j * 2 + p) * SK + kt
                if kt % 4 == 0:
                    pop_seq_now(("k", p, kt // 4))
                # past-deadline fillers + pacing extras
                scheduler_pop(astep, 1)
                # drain ready ctx work (vsb emitted, exp old enough);
                # deeper lag at pair boundaries so the previous pair's
                # last ctx (waiting on its exp) cannot delay the new
                # pair's scores in the PE stream
                lag = 2 if kt < 2 else 1
                # hold a few ctx drains back during j0 so the
                # projection-heavy phase spills into j1's ACT slack
                thresh = max(0, 4 - max(0, astep - 31))
                while pending and pending[0][3] <= astep - lag and \
                        pending[0][1] in vsb_emitted and \
                        len(pending) > thresh:
                    drain_one()
                # backlog guard for the ex ring
                while len(pending) >= EXBUFS - 2:
                    drain_one()
                # scores (row-tiled pair) + exp; high priority keeps the
                # T0/T8 pair adjacent in the PE stream so they overlap
                sc = sc_ps.tile([P, 1024], f32, tag="sc", name="sc")
                with tc.high_priority(offset=10 ** 6):
                    for i in range(2):
                        nc.tensor.matmul(
                            sc[:, i * 512:(i + 1) * 512],
                            kT[p][i * HD:(i + 1) * HD, kt * P:(kt + 1) * P],
                            qT[p][i * HD:(i + 1) * HD, j * 512:(j + 1) * 512],
                            start=True, stop=True)
                ex = exq.tile([P, 1024], bf16, tag="ex", name="ex")
                nc.scalar.activation(ex[:], sc[:], EXP, scale=SCALE)
                pending.append(((j * 2 + p), kt, ex, astep))

    # tail: drain everything, then remaining fillers (outproj j=3)
    while pending:
        drain_one()
    close_open()
    while fillers:
        key = min(fillers.keys(),
                  key=lambda k: (fillers[k][0], str(k[0]), k[1]))
        _pop(*key)


_CACHE = {}


def _build():
    if "nc" in _CACHE:
        return _CACHE["nc"]
    nc = bacc.Bacc("TRN2", target_bir_lowering=False, debug=False)
    QTd = nc.dram_tensor("QT", [P, DK * S], bf16, kind="ExternalInput").ap()
    KTd = nc.dram_tensor("KT", [P, DK * S], bf16, kind="ExternalInput").ap()
    VTd = nc.dram_tensor("VT", [P, DK * S], bf16, kind="ExternalInput").ap()
    WqT = nc.dram_tensor("WqT", [P, DK * FS], bf16, kind="ExternalInput").ap()
    WkT = nc.dram_tensor("WkT", [P, DK * FS], bf16, kind="ExternalInput").ap()
    WvT = nc.dram_tensor("WvT", [P, DK * FS], bf16, kind="ExternalInput").ap()
    WoT = nc.dram_tensor("WoT", [P, 2 * D], bf16, kind="ExternalInput").ap()
    bq = nc.dram_tensor("bq", [P, 2], f32, kind="ExternalInput").ap()
    bk = nc.dram_tensor("bk", [P, 2], f32, kind="ExternalInput").ap()
    bv = nc.dram_tensor("bv", [1, FS], f32, kind="ExternalInput").ap()
    OUTP = nc.dram_tensor("OUTP", [S, D], bf16, kind="ExternalOutput").ap()
    with tile.TileContext(nc) as tc, ExitStack() as ctx:
        _emit(ctx, tc, nc,
              (QTd, KTd, VTd, WqT, WkT, WvT, WoT, bq, bk, bv, OUTP))
    nc.compile()
    _CACHE["nc"] = nc
    return nc


def _in_maps(Q, K, V, Wq, bq, Wk, bk, Wv, bv, Wo, bo):
    bf = ml_dtypes.bfloat16

    def packS(x):
        # x [S, D] activation -> xT [D, S] -> [p, q, k, c] quarter-major
        a = np.asarray(x, np.float32).T.astype(bf)          # [D, S]
        a = a.reshape(DK, P, NQ, 512).transpose(1, 2, 0, 3)
        return np.ascontiguousarray(a).reshape(P, DK * S)

    def packW(wT):
        # wT [D_in, F_out] -> [p, k, f]
        a = np.asarray(wT).astype(bf)
        k = a.shape[0] // P
        a = a.reshape(k, P, a.shape[1]).transpose(1, 0, 2)
        return np.ascontiguousarray(a).reshape(P, -1)

    QTb = [packS(Q[b]) for b in range(B)]
    KTb = [packS(K[b]) for b in range(B)]
    VTb = [packS(V[b]) for b in range(B)]
    c = np.ascontiguousarray
    maps = []
    for core in range(8):
        b, g = divmod(core, NG)
        sl = slice(g * FS, (g + 1) * FS)
        maps.append({
            "QT": QTb[b], "KT": KTb[b], "VT": VTb[b],
            "WqT": packW(np.asarray(Wq)[sl, :].T),
            "WkT": packW(np.asarray(Wk)[sl, :].T),
            "WvT": packW(np.asarray(Wv)[sl, :].T),
            "WoT": packW(np.asarray(Wo)[:, sl].T),
            "bq": c(np.asarray(bq, np.float32)[sl].reshape(2, P).T),
            "bk": c(np.asarray(bk, np.float32)[sl].reshape(2, P).T),
            "bv": c(np.asarray(bv, np.float32)[sl].reshape(1, FS)),
        })
    return maps


def kernel(Q, K, V, Wq, bq, Wk, bk, Wv, bv, Wo, bo):
    nc = _build()
    maps = _in_maps(Q, K, V, Wq, bq, Wk, bk, Wv, bv, Wo, bo)
    res = run_bass_kernel_spmd(nc, maps, core_ids=list(range(8)))
    out = np.empty((B, S, D), np.float32)
    for b in range(B):
        acc = res.results[b * NG]["OUTP"].astype(np.float32)
        for g in range(1, NG):
            acc = acc + res.results[b * NG + g]["OUTP"].astype(np.float32)
        out[b] = acc + np.asarray(bo, np.float32)[None, :]
    return out


# revision 52
# speedup vs baseline: 1.1704x; 1.1704x over previous
"""Trainium2 Bass kernel for nn_MultiHeadAttention (B=2, S=2048, D=1024, H=16).

Sharding (8 cores): data-parallel over batch (2) x tensor-parallel over
head groups (4 groups of 4 heads).  Core c handles batch c//4, heads
4*(c%4) .. 4*(c%4)+3 plus its slice of the output projection; the host
sums the 4 partial output projections per batch and adds bo.

Design notes (~253us baseline -> ~218-220us):
  * scores matmuls run ROW-TILED (K=64 head dim -> tile_size (64,128)):
    the two heads of a pair live on SBUF partitions 0-63 / 64-127, so
    their score matmuls land on PE row-tiles T0/T8 and stream
    CONCURRENTLY (measured ~118ns/MM vs 215 serial).  They are emitted
    under tc.high_priority so the tile scheduler keeps the pair
    adjacent in the PE stream (it otherwise splits them).
  * one exp() activation per (pair, key tile) covers both heads
    ([128,1024] PSUM -> bf16 SBUF, ~1.1us/call, 128 calls ~= 134us
    busy); exp is the #2 engine after the PE (~185us streaming work).
  * PSUM: 2x sc [128,1024] (4 banks) + 2x ctx accum (2) + 2x proj
    accum (2) = 8 banks exactly; ctx accumulates v'@ex over 16 key
    tiles with a fused ones-column giving the softmax denominators.
  * q/k/v/out projections are deadline-scheduled filler granules popped
    between attention steps; ctx matmuls drain lagged behind exp so the
    PE never waits on the activation right before a scores pair.
  * inputs are HOST-PACKED per partition (contiguous DMA rows; the
    naive [p,k,c] gather measured only ~85GB/s) and stream in
    need-order chains with the six criticals split k-halves across
    sync+gpsimd (a single queue caps at ~110-135GB/s; bulk stays on
    sync only -- bulk on gpsimd blocks its queue and delays the norm
    broadcasts, measured +7us).
  * outproj j>=2 is reserved for the tail with accumulators spread
    over the freed sc banks and copies split across scalar/vector;
    output is written bf16 (host sums the 4 partials in fp32).
"""

import sys

for _p in ("/opt/trn_rl_repo",):
    if _p not in sys.path:
        sys.path.insert(0, _p)

from contextlib import ExitStack

import ml_dtypes
import numpy as np

import concourse.bass as bass
import concourse.tile as tile
from concourse import bacc, mybir
from concourse.bass_utils import run_bass_kernel_spmd

B, S, D, H = 2, 2048, 1024, 16
HD = D // H            # 64 head dim
NG = 4                 # head groups (cores per batch)
NHC = H // NG          # 4 heads per core
FS = NHC * HD          # 256 features per core
P = 128
DK = D // P            # 8 contraction tiles for projections
SK = S // P            # 16 key tiles
NQ = S // 512          # 4 query chunks
VW = HD + 1            # v feats + ones column

f32 = mybir.dt.float32
bf16 = mybir.dt.bfloat16
EXP = mybir.ActivationFunctionType.Exp
EXBUFS = 12            # ex tile ring (must exceed max ctx-drain backlog)
SCALE = 1.0 / (HD ** 0.5)


def _emit(ctx: ExitStack, tc, nc, io):
    QT, KT, VT, WqT, WkT, WvT, WoT, bq, bk, bv, OUTP = io

    # ---- pools (PSUM pools first => bank-aligned slots) ----
    sc_ps = ctx.enter_context(tc.tile_pool(name="sc_ps", bufs=2, space="PSUM"))
    ctx_ps = ctx.enter_context(tc.tile_pool(name="ctx_ps", bufs=2, space="PSUM"))
    acc_ps = ctx.enter_context(tc.tile_pool(name="acc_ps", bufs=2, space="PSUM"))
    wp = ctx.enter_context(tc.tile_pool(name="wp", bufs=1))
    per = ctx.enter_context(tc.tile_pool(name="per", bufs=1))
    exq = ctx.enter_context(tc.tile_pool(name="exq", bufs=EXBUFS))
    nrm = ctx.enter_context(tc.tile_pool(name="nrm", bufs=2))
    cnp = ctx.enter_context(tc.tile_pool(name="cnp", bufs=2))
    outp = ctx.enter_context(tc.tile_pool(name="outp", bufs=3))

    # ---- persistent SBUF ----
    wk_all = wp.tile([P, DK * FS], bf16, tag="wk")   # [p, (k, fs)]
    wq_all = wp.tile([P, DK * FS], bf16, tag="wq")
    wv_all = wp.tile([P, DK * FS], bf16, tag="wv")
    wo_all = wp.tile([P, 2 * D], bf16, tag="wo")     # [p, (f, d)]
    bqt = wp.tile([P, 2], f32, tag="bqt")            # [p, f]
    bkt = wp.tile([P, 2], f32, tag="bkt")
    bvt = wp.tile([P, FS], f32, tag="bvt")
    KTi = wp.tile([P, DK * S], bf16, tag="KTi")      # [p, (k, c)]
    QTi = wp.tile([P, DK * S], bf16, tag="QTi")
    VTi = wp.tile([P, DK * S], bf16, tag="VTi")
    kT = [per.tile([P, S], bf16, tag=f"kT{f}", name=f"kT{f}") for f in range(2)]
    qT = [per.tile([P, S], bf16, tag=f"qT{f}", name=f"qT{f}") for f in range(2)]
    vsb = [per.tile([P, NHC * VW], bf16, tag=f"v{t}", name=f"v{t}")
           for t in range(SK)]

    KTi3 = KTi.rearrange("p (k c) -> p k c", c=S)
    QTi3 = QTi.rearrange("p (k c) -> p k c", c=S)
    VTi3 = VTi.rearrange("p (k c) -> p k c", c=S)

    # ---- exp table load (cold matmul warm-up is counterproductive:
    # cold MMs run at ~50% duty and never trip HAM; dense projection
    # work warms the PE in ~3.4us on its own) ----
    warm_sb = wp.tile([P, 16], bf16, tag="warm")
    nc.vector.memset(warm_sb[:], 0.0)
    warm_ex = wp.tile([P, 16], bf16, tag="warmex")
    nc.scalar.activation(warm_ex[:], warm_sb[:], EXP, scale=0.125)

    # ---- input DMAs, priority order ----
    def qslice(dram3, q):
        return dram3[:, :, q * 512:(q + 1) * 512]


    # Inputs are HOST-PACKED so every DMA is contiguous per partition
    # (the [p, k, c] gather pattern measured only ~85 GB/s; contiguous
    # rows run at full HBM rate).  Seq tensors are packed quarter-major:
    # dram[p, q, k, c] = XT[k*128+p, q*512+c].
    # K criticals first and ALONE at full bandwidth (sync chain); the Q
    # chain (gpsimd) is gated behind KTq0 by a dummy copy dep; V + bulk
    # chain behind K on sync; biases on scalar (tiny).
    def qsrc(dram, q):
        return dram[:, q * 4096:(q + 1) * 4096].rearrange(
            "p (k c) -> p k c", c=512)

    nc.scalar.dma_start(bkt[:], bk[:, :])
    nc.scalar.dma_start(bqt[:], bq[:, :])
    nc.scalar.dma_start(bvt[:], bv.to_broadcast((P, FS)))
    # criticals split k-halves across sync+gpsimd (a single queue moves
    # only ~110-135 GB/s total); each queue's chain is in need-order so
    # in-queue descriptor sequencing keeps the priority
    def crit(dst3, src3):
        nc.sync.dma_start(dst3[:, 0:4], src3[:, 0:4])
        nc.gpsimd.dma_start(dst3[:, 4:8], src3[:, 4:8])

    def w3(dst, src):
        return (dst.rearrange("p (k c) -> p k c", c=FS),
                src.rearrange("p (k c) -> p k c", c=FS))

    crit(*w3(wk_all, WkT))
    crit(qslice(KTi3, 0), qsrc(KT, 0))
    crit(*w3(wq_all, WqT))
    crit(qslice(QTi3, 0), qsrc(QT, 0))
    crit(*w3(wv_all, WvT))
    crit(qslice(VTi3, 0), qsrc(VT, 0))
    for q in (1, 2, 3):
        nc.sync.dma_start(qslice(KTi3, q), qsrc(KT, q))
        nc.sync.dma_start(qslice(VTi3, q), qsrc(VT, q))
    nc.sync.dma_start(wo_all[:], WoT[:, :])
    for q in (1, 2, 3):
        nc.sync.dma_start(qslice(QTi3, q), qsrc(QT, q))

    # ================= filler granules =================
    # Each projection quarter is a 2-granule sequence [open, close] over
    # one acc_ps accumulator; at most 2 sequences may be open at a time.
    kq_state = {}

    def kq_granule(dst, w_all, b_t, src3, f, q, part, label=""):
        def g():
            key = (label, f, q)
            if part == 0:
                kq_state[key] = acc_ps.tile([P, 512], f32, tag="acc",
                                            name="pacc")
            ps = kq_state.pop(key) if part == 1 else kq_state[key]
            for k in range(4 * part, 4 * part + 4):
                nc.tensor.matmul(
                    ps[:], w_all[:, k * FS + f * P: k * FS + (f + 1) * P],
                    src3[:, k, q * 512:(q + 1) * 512],
                    start=(k == 0), stop=(k == DK - 1))
            if part == 1:
                nc.vector.tensor_scalar_add(
                    dst[f][:, q * 512:(q + 1) * 512], ps[:], b_t[:, f:f + 1])
        return g

    v_state = {}

    def v_granule(t, part):
        def g():
            if part == 0:
                v_state[t] = acc_ps.tile([P, FS], f32, tag="acc", name="vacc")
            ps = v_state.pop(t) if part == 1 else v_state[t]
            for k in range(4 * part, 4 * part + 4):
                nc.tensor.matmul(
                    ps[:], VTi3[:, k, t * P:(t + 1) * P],
                    wv_all[:, k * FS:(k + 1) * FS],
                    start=(k == 0), stop=(k == DK - 1))
            if part == 1:
                v3 = vsb[t].rearrange("p (h w) -> p h w", w=VW)
                nc.vector.tensor_add(
                    v3[:, :, 0:HD],
                    ps.rearrange("p (h w) -> p h w", w=HD),
                    bvt.rearrange("p (h w) -> p h w", w=HD))
                nc.vector.memset(v3[:, :, HD:VW], 1.0)
        return g

    def o_granule(j, mt, oc, cn, ob_box, eng=None, use_sc=False):
        def g():
            if oc == 0:
                ob_box.append(outp.tile([P, 1024], bf16, tag="ob", name="ob"))
            ob = ob_box[-1]
            if use_sc:
                # tail only: scores are done, reuse the sc PSUM banks so
                # the outproj accumulators rotate over 4 banks
                big = sc_ps.tile([P, 1024], f32, tag="sc", name="oacc2")
                ps = big[:, 0:512]
            else:
                ps = acc_ps.tile([P, 512], f32, tag="acc", name="oacc")
            for f in range(2):
                nc.tensor.matmul(
                    ps[:], cn[f][:, mt * P:(mt + 1) * P],
                    wo_all[:, f * D + oc * 512: f * D + (oc + 1) * 512],
                    start=(f == 0), stop=(f == 1))
            dst = ob[:, oc * 512:(oc + 1) * 512]
            if eng == "scalar":
                nc.scalar.copy(dst, ps[:])
            elif eng == "gpsimd":
                nc.gpsimd.tensor_copy(dst, ps[:])
            else:
                nc.vector.tensor_copy(dst, ps[:])
            if oc == 1:
                nc.gpsimd.dma_start(
                    OUTP[j * 512 + mt * P: j * 512 + (mt + 1) * P, :], ob[:])
        return g

    # ---- filler bookkeeping ----
    # fillers[(sid, part)] = [deadline, earliest, fn].  Sequences of kind
    # k/q/v share one acc_ps accumulator across their two granules; at
    # most ONE such sequence may be open (part 0 popped, part 1 not).
    fillers = {}
    state = {"open": None}
    vsb_emitted = set()

    def add_seq(sid, earliest, deadline, fns):
        for part, fn in enumerate(fns):
            fillers[(sid, part)] = [deadline, earliest, fn]

    def _pop(sid, part):
        ent = fillers.pop((sid, part), None)
        if ent is None:
            return False
        ent[2]()
        if sid[0] in ("k", "q", "v"):
            state["open"] = sid if part == 0 else None
        if sid[0] == "v" and part == 1:
            vsb_emitted.add(sid[1])
        return True

    def close_open():
        if state["open"] is not None:
            _pop(state["open"], 1)

    def pop_seq_now(sid):
        if state["open"] is not None and state["open"] != sid:
            close_open()
        _pop(sid, 0)
        _pop(sid, 1)

    def scheduler_pop(astep, budget):
        # pop all past-deadline granules (free) + up to `budget` extras
        spent = 0
        while True:
            if state["open"] is not None:
                sid = state["open"]
                ent = fillers.get((sid, 1))
                due = ent is not None and ent[0] <= astep
                if not due and spent >= budget:
                    return
                _pop(sid, 1)
                if not due:
                    spent += 1
                continue
            best = None
            for (sid, part), ent in fillers.items():
                if part == 1 and (sid, 0) in fillers:
                    continue
                due = ent[0] <= astep
                if not due and ent[1] > astep:
                    continue
                key = (0 if due else 1, ent[0], ent[1], str(sid))
                if best is None or key < best[0]:
                    best = (key, sid, part, due)
            if best is None:
                return
            if not best[3] and spent >= budget:
                return
            _pop(best[1], best[2])
            if not best[3]:
                spent += 1

    # register filler sequences
    # kproj quarters: f is the pair index; scores (j=0,p,kt) need q=kt//4
    for f in range(2):
        for q in range(4):
            first_use = f * SK + 4 * q
            add_seq(("k", f, q), max(0, first_use - 8), first_use - 3,
                    [kq_granule(kT, wk_all, bkt, KTi3, f, q, p2, "k")
                     for p2 in range(2)])
    # qproj: qT[f] quarter j needed at astep (j*2+f)*SK
    for f in range(2):
        for j in range(NQ):
            first_use = (j * 2 + f) * SK
            add_seq(("q", f, j), max(0, first_use - 10), first_use - 4,
                    [kq_granule(qT, wq_all, bqt, QTi3, f, j, p2, "q")
                     for p2 in range(2)])
    # vproj: vsb[t] needed by ctx drain of (j=0, p=0, kt=t)
    for t in range(SK):
        add_seq(("v", t), max(0, t - 4), t,
                [v_granule(t, p2) for p2 in range(2)])

    # ================= attention =================
    pending = []           # (pair_serial, kt, ex, emit_astep)
    pair_cp = {}           # pair_serial -> [cp_even, cp_odd]
    pair_drained = {}
    ndrained = 0
    cn_byj = {}

    def drain_one():
        nonlocal ndrained
        ps_serial, kt, ex, _ = pending.pop(0)
        j, p = divmod(ps_serial, 2)
        if kt not in vsb_emitted:
            pop_seq_now(("v", kt))
        if ps_serial not in pair_cp:
            pair_cp[ps_serial] = [
                ctx_ps.tile([VW, 512], f32, tag="ctx", name=f"cp{ps_serial}h{i}")
                for i in range(2)]
            pair_drained[ps_serial] = 0
        cps = pair_cp[ps_serial]
        nd = pair_drained[ps_serial]
        for i in range(2):
            h = 2 * p + i
            nc.tensor.matmul(
                cps[i][:], vsb[kt][:, h * VW:(h + 1) * VW],
                ex[:, i * 512:(i + 1) * 512],
                start=(nd == 0), stop=(nd == SK - 1))
        pair_drained[ps_serial] = nd + 1
        ndrained += 1
        if nd == SK - 1:
            finish_pair(ps_serial)

    def finish_pair(ps_serial):
        j, p = divmod(ps_serial, 2)
        last = ps_serial == 2 * NQ - 1
        if j not in cn_byj:
            cn_byj[j] = [cnp.tile([P, 512], bf16, tag=f"cn{f}", name=f"cn{f}")
                         for f in range(2)]
        cn = cn_byj[j]
        cps = pair_cp.pop(ps_serial)
        for i in range(2):
            h = 2 * p + i
            if last:
                # tail: no PSUM pressure; skip staging, shortest chain
                src, srcsum = cps[i], cps[i][HD:HD + 1, :]
            else:
                cu = nrm.tile([VW, 512], f32, tag="cu", name="cu")
                with tc.high_priority(offset=5 * 10 ** 5):
                    nc.vector.tensor_copy(cu[:], cps[i][:])  # frees PSUM
                src, srcsum = cu, cu[HD:HD + 1, :]
            sm = nrm.tile([1, 512], f32, tag="sm", name="sm")
            nc.vector.tensor_copy(sm[:], srcsum)
            r1 = nrm.tile([1, 512], f32, tag="r1", name="r1")
            t1 = nrm.tile([1, 512], f32, tag="t1", name="t1")
            nc.vector.reciprocal_approx_accurate(r1[:], sm[:], t1[:])
            rb = nrm.tile([HD, 512], f32, tag="rb", name="rb")
            nc.gpsimd.partition_broadcast(rb[:], r1[:])
            nc.vector.tensor_mul(cn[p][(i) * HD:(i + 1) * HD, :],
                                 src[0:HD, :], rb[:])
        if p == 1:
            # both pairs of qchunk j normed -> register outproj fillers.
            # j>=2 outproj is reserved for the tail (keeps the PE warm
            # while the final norm chain runs).
            here = ps_serial * SK + SK - 1
            tail_j = j >= NQ - 2
            dl = 10 ** 9 if tail_j else (j + 1) * 2 * SK + 24
            early = 10 ** 9 if tail_j else here + 2
            for mt in range(4):
                ob_box = []
                eng = ("scalar", "vector") if tail_j else (None, None)
                add_seq(("o", j, mt), early, dl,
                        [o_granule(j, mt, oc, cn, ob_box, eng=eng[oc],
                                   use_sc=tail_j and (mt * 2 + oc) % 2 == 0)
                         for oc in (0, 1)])

    for j in range(NQ):
        for p in range(2):
            if j == 0:
                pop_seq_now(("k", p, 0))   # kproj first: K data lands first
            pop_seq_now(("q", p, j))
            for kt in range(SK):
                astep = (j * 2 + p) * SK + kt
                if kt % 4 == 0:
                    pop_seq_now(("k", p, kt // 4))
                # past-deadline fillers + pacing extras
                scheduler_pop(astep, 1)
                # drain ready ctx work (vsb emitted, exp old enough);
                # deeper lag at pair boundaries so the previous pair's
                # last ctx (waiting on its exp) cannot delay the new
                # pair's scores in the PE stream
                lag = 2 if kt < 2 else 1
                while pending and pending[0][3] <= astep - lag and \
                        pending[0][1] in vsb_emitted:
                    drain_one()
                # backlog guard for the ex ring
                while len(pending) >= EXBUFS - 2:
                    drain_one()
                # scores (row-tiled pair) + exp; high priority keeps the
                # T0/T8 pair adjacent in the PE stream so they overlap
                sc = sc_ps.tile([P, 1024], f32, tag="sc", name="sc")
                with tc.high_priority(offset=10 ** 6):
                    for i in range(2):
                        nc.tensor.matmul(
                            sc[:, i * 512:(i + 1) * 512],
                            kT[p][i * HD:(i + 1) * HD, kt * P:(kt + 1) * P],
                            qT[p][i * HD:(i + 1) * HD, j * 512:(j + 1) * 512],
                            start=True, stop=True)
                ex = exq.tile([P, 1024], bf16, tag="ex", name="ex")
                nc.scalar.activation(ex[:], sc[:], EXP, scale=SCALE)
                pending.append(((j * 2 + p), kt, ex, astep))

    # tail: drain everything, then remaining fillers (outproj j=3)
    while pending:
        drain_one()
    close_open()
    while fillers:
        key = min(fillers.keys(),
                  key=lambda k: (fillers[k][0], str(k[0]), k[1]))
        _pop(*key)


_CACHE = {}


def _build():
    if "nc" in _CACHE:
        return _CACHE["nc"]
    nc = bacc.Bacc("TRN2", target_bir_lowering=False, debug=False)
    QTd = nc.dram_tensor("QT", [P, DK * S], bf16, kind="ExternalInput").ap()
    KTd = nc.dram_tensor("KT", [P, DK * S], bf16, kind="ExternalInput").ap()
    VTd = nc.dram_tensor("VT", [P, DK * S], bf16, kind="ExternalInput").ap()
    WqT = nc.dram_tensor("WqT", [P, DK * FS], bf16, kind="ExternalInput").ap()
    WkT = nc.dram_tensor("WkT", [P, DK * FS], bf16, kind="ExternalInput").ap()
    WvT = nc.dram_tensor("WvT", [P, DK * FS], bf16, kind="ExternalInput").ap()
    WoT = nc.dram_tensor("WoT", [P, 2 * D], bf16, kind="ExternalInput").ap()
    bq = nc.dram_tensor("bq", [P, 2], f32, kind="ExternalInput").ap()
    bk = nc.dram_tensor("bk", [P, 2], f32, kind="ExternalInput").ap()
    bv = nc.dram_tensor("bv", [1, FS], f32, kind="ExternalInput").ap()
    OUTP = nc.dram_tensor("OUTP", [S, D], bf16, kind="ExternalOutput").ap()
    with tile.TileContext(nc) as tc, ExitStack() as ctx:
        _emit(ctx, tc, nc,
              (QTd, KTd, VTd, WqT, WkT, WvT, WoT, bq, bk, bv, OUTP))
    nc.compile()
    _CACHE["nc"] = nc
    return nc


def _in_maps(Q, K, V, Wq, bq, Wk, bk, Wv, bv, Wo, bo):
    bf = ml_dtypes.bfloat16

    def packS(x):
        # x [S, D] activation -> xT [D, S] -> [p, q, k, c] quarter-major
        a = np.asarray(x, np.float32).T.astype(bf)          # [D, S]
        a = a.reshape(DK, P, NQ, 512).transpose(1, 2, 0, 3)
        return np.ascontiguousarray(a).reshape(P, DK * S)

    def packW(wT):
        # wT [D_in, F_out] -> [p, k, f]
        a = np.asarray(wT).astype(bf)
        k = a.shape[0] // P
        a = a.reshape(k, P, a.shape[1]).transpose(1, 0, 2)
        return np.ascontiguousarray(a).reshape(P, -1)

    QTb = [packS(Q[b]) for b in range(B)]
    KTb = [packS(K[b]) for b in range(B)]
    VTb = [packS(V[b]) for b in range(B)]
    c = np.ascontiguousarray
    maps = []
    for core in range(8):
        b, g = divmod(core, NG)
        sl = slice(g * FS, (g + 1) * FS)
        maps.append({
            "QT": QTb[b], "KT": KTb[b], "VT": VTb[b],
            "WqT": packW(np.asarray(Wq)[sl, :].T),
            "WkT": packW(np.asarray(Wk)[sl, :].T),
            "WvT": packW(np.asarray(Wv)[sl, :].T),
            "WoT": packW(np.asarray(Wo)[:, sl].T),
            "bq": c(np.asarray(bq, np.float32)[sl].reshape(2, P).T),
            "bk": c(np.asarray(bk, np.float32)[sl].reshape(2, P).T),
            "bv": c(np.asarray(bv, np.float32)[sl].reshape(1, FS)),
        })
    return maps


def kernel(Q, K, V, Wq, bq, Wk, bk, Wv, bv, Wo, bo):
    nc = _build()
    maps = _in_maps(Q, K, V, Wq, bq, Wk, bk, Wv, bv, Wo, bo)
    res = run_bass_kernel_spmd(nc, maps, core_ids=list(range(8)))
    out = np.empty((B, S, D), np.float32)
    for b in range(B):
        acc = res.results[b * NG]["OUTP"].astype(np.float32)
        for g in range(1, NG):
            acc = acc + res.results[b * NG + g]["OUTP"].astype(np.float32)
        out[b] = acc + np.asarray(bo, np.float32)[None, :]
    return out


# revision 54
# speedup vs baseline: 1.1773x; 1.0058x over previous
"""Trainium2 Bass kernel for nn_MultiHeadAttention (B=2, S=2048, D=1024, H=16).

Sharding (8 cores): data-parallel over batch (2) x tensor-parallel over
head groups (4 groups of 4 heads).  Core c handles batch c//4, heads
4*(c%4) .. 4*(c%4)+3 plus its slice of the output projection; the host
sums the 4 partial output projections per batch and adds bo.

Design notes (~253us baseline -> ~215-218us):
  * scores matmuls run ROW-TILED (K=64 head dim -> tile_size (64,128)):
    the two heads of a pair live on SBUF partitions 0-63 / 64-127, so
    their score matmuls land on PE row-tiles T0/T8 and stream
    CONCURRENTLY (measured ~118ns/MM vs 215 serial).  They are emitted
    under tc.high_priority so the tile scheduler keeps the pair
    adjacent in the PE stream (it otherwise splits them).
  * one exp() activation per (pair, key tile) covers both heads
    ([128,1024] PSUM -> bf16 SBUF, ~1.1us/call, 128 calls ~= 134us
    busy); exp is the #2 engine after the PE (~185us streaming work).
  * PSUM: 2x sc [128,1024] (4 banks) + 2x ctx accum (2) + 2x proj
    accum (2) = 8 banks exactly; ctx accumulates v'@ex over 16 key
    tiles with a fused ones-column giving the softmax denominators.
  * q/k/v/out projections are deadline-scheduled filler granules popped
    between attention steps; ctx matmuls drain lagged behind exp so the
    PE never waits on the activation right before a scores pair.
  * inputs are HOST-PACKED per partition (contiguous DMA rows; the
    naive [p,k,c] gather measured only ~85GB/s) and stream in
    need-order chains with the six criticals split k-halves across
    sync+gpsimd (a single queue caps at ~110-135GB/s; bulk stays on
    sync only -- bulk on gpsimd blocks its queue and delays the norm
    broadcasts, measured +7us).
  * outproj j>=2 is reserved for the tail with accumulators spread
    over the freed sc banks and copies split across scalar/vector;
    output is written bf16 (host sums the 4 partials in fp32).
"""

import sys

for _p in ("/opt/trn_rl_repo",):
    if _p not in sys.path:
        sys.path.insert(0, _p)

from contextlib import ExitStack

import ml_dtypes
import numpy as np

import concourse.bass as bass
import concourse.tile as tile
from concourse import bacc, mybir
from concourse.bass_utils import run_bass_kernel_spmd

B, S, D, H = 2, 2048, 1024, 16
HD = D // H            # 64 head dim
NG = 4                 # head groups (cores per batch)
NHC = H // NG          # 4 heads per core
FS = NHC * HD          # 256 features per core
P = 128
DK = D // P            # 8 contraction tiles for projections
SK = S // P            # 16 key tiles
NQ = S // 512          # 4 query chunks
VW = HD + 1            # v feats + ones column

f32 = mybir.dt.float32
bf16 = mybir.dt.bfloat16
EXP = mybir.ActivationFunctionType.Exp
EXBUFS = 12            # ex tile ring (must exceed max ctx-drain backlog)
SCALE = 1.0 / (HD ** 0.5)


def _emit(ctx: ExitStack, tc, nc, io):
    QT, KT, VT, WqT, WkT, WvT, WoT, bq, bk, bv, OUTP = io

    # ---- pools (PSUM pools first => bank-aligned slots) ----
    sc_ps = ctx.enter_context(tc.tile_pool(name="sc_ps", bufs=2, space="PSUM"))
    ctx_ps = ctx.enter_context(tc.tile_pool(name="ctx_ps", bufs=2, space="PSUM"))
    acc_ps = ctx.enter_context(tc.tile_pool(name="acc_ps", bufs=2, space="PSUM"))
    wp = ctx.enter_context(tc.tile_pool(name="wp", bufs=1))
    per = ctx.enter_context(tc.tile_pool(name="per", bufs=1))
    exq = ctx.enter_context(tc.tile_pool(name="exq", bufs=EXBUFS))
    nrm = ctx.enter_context(tc.tile_pool(name="nrm", bufs=2))
    cnp = ctx.enter_context(tc.tile_pool(name="cnp", bufs=2))
    outp = ctx.enter_context(tc.tile_pool(name="outp", bufs=3))

    # ---- persistent SBUF ----
    wk_all = wp.tile([P, DK * FS], bf16, tag="wk")   # [p, (k, fs)]
    wq_all = wp.tile([P, DK * FS], bf16, tag="wq")
    wv_all = wp.tile([P, DK * FS], bf16, tag="wv")
    wo_all = wp.tile([P, 2 * D], bf16, tag="wo")     # [p, (f, d)]
    bqt = wp.tile([P, 2], f32, tag="bqt")            # [p, f]
    bkt = wp.tile([P, 2], f32, tag="bkt")
    bvt = wp.tile([P, FS], f32, tag="bvt")
    KTi = wp.tile([P, DK * S], bf16, tag="KTi")      # [p, (k, c)]
    QTi = wp.tile([P, DK * S], bf16, tag="QTi")
    VTi = wp.tile([P, DK * S], bf16, tag="VTi")
    kT = [per.tile([P, S], bf16, tag=f"kT{f}", name=f"kT{f}") for f in range(2)]
    qT = [per.tile([P, S], bf16, tag=f"qT{f}", name=f"qT{f}") for f in range(2)]
    vsb = [per.tile([P, NHC * VW], bf16, tag=f"v{t}", name=f"v{t}")
           for t in range(SK)]

    KTi3 = KTi.rearrange("p (k c) -> p k c", c=S)
    QTi3 = QTi.rearrange("p (k c) -> p k c", c=S)
    VTi3 = VTi.rearrange("p (k c) -> p k c", c=S)

    # ---- exp table load (cold matmul warm-up is counterproductive:
    # cold MMs run at ~50% duty and never trip HAM; dense projection
    # work warms the PE in ~3.4us on its own) ----
    warm_sb = wp.tile([P, 16], bf16, tag="warm")
    nc.vector.memset(warm_sb[:], 0.0)
    warm_ex = wp.tile([P, 16], bf16, tag="warmex")
    nc.scalar.activation(warm_ex[:], warm_sb[:], EXP, scale=0.125)

    # ---- input DMAs, priority order ----
    def qslice(dram3, q):
        return dram3[:, :, q * 512:(q + 1) * 512]


    # Inputs are HOST-PACKED so every DMA is contiguous per partition
    # (the [p, k, c] gather pattern measured only ~85 GB/s; contiguous
    # rows run at full HBM rate).  Seq tensors are packed quarter-major:
    # dram[p, q, k, c] = XT[k*128+p, q*512+c].
    # K criticals first and ALONE at full bandwidth (sync chain); the Q
    # chain (gpsimd) is gated behind KTq0 by a dummy copy dep; V + bulk
    # chain behind K on sync; biases on scalar (tiny).
    def qsrc(dram, q):
        return dram[:, q * 4096:(q + 1) * 4096].rearrange(
            "p (k c) -> p k c", c=512)

    nc.scalar.dma_start(bkt[:], bk[:, :])
    nc.scalar.dma_start(bqt[:], bq[:, :])
    nc.scalar.dma_start(bvt[:], bv.to_broadcast((P, FS)))
    # criticals split k-halves across sync+gpsimd (a single queue moves
    # only ~110-135 GB/s total); each queue's chain is in need-order so
    # in-queue descriptor sequencing keeps the priority
    def crit(dst3, src3):
        nc.sync.dma_start(dst3[:, 0:4], src3[:, 0:4])
        nc.gpsimd.dma_start(dst3[:, 4:8], src3[:, 4:8])

    def w3(dst, src):
        return (dst.rearrange("p (k c) -> p k c", c=FS),
                src.rearrange("p (k c) -> p k c", c=FS))

    crit(*w3(wk_all, WkT))
    crit(qslice(KTi3, 0), qsrc(KT, 0))
    crit(*w3(wq_all, WqT))
    crit(qslice(QTi3, 0), qsrc(QT, 0))
    crit(*w3(wv_all, WvT))
    crit(qslice(VTi3, 0), qsrc(VT, 0))
    for q in (1, 2, 3):
        nc.sync.dma_start(qslice(KTi3, q), qsrc(KT, q))
        nc.sync.dma_start(qslice(VTi3, q), qsrc(VT, q))
    nc.sync.dma_start(wo_all[:], WoT[:, :])
    for q in (1, 2, 3):
        nc.sync.dma_start(qslice(QTi3, q), qsrc(QT, q))

    # ================= filler granules =================
    # Each projection quarter is a 2-granule sequence [open, close] over
    # one acc_ps accumulator; at most 2 sequences may be open at a time.
    kq_state = {}

    def kq_granule(dst, w_all, b_t, src3, f, q, part, label=""):
        def g():
            key = (label, f, q)
            if part == 0:
                kq_state[key] = acc_ps.tile([P, 512], f32, tag="acc",
                                            name="pacc")
            ps = kq_state.pop(key) if part == 1 else kq_state[key]
            for k in range(4 * part, 4 * part + 4):
                nc.tensor.matmul(
                    ps[:], w_all[:, k * FS + f * P: k * FS + (f + 1) * P],
                    src3[:, k, q * 512:(q + 1) * 512],
                    start=(k == 0), stop=(k == DK - 1))
            if part == 1:
                nc.vector.tensor_scalar_add(
                    dst[f][:, q * 512:(q + 1) * 512], ps[:], b_t[:, f:f + 1])
        return g

    v_state = {}

    def v_granule(t, part):
        def g():
            if part == 0:
                v_state[t] = acc_ps.tile([P, FS], f32, tag="acc", name="vacc")
            ps = v_state.pop(t) if part == 1 else v_state[t]
            for k in range(4 * part, 4 * part + 4):
                nc.tensor.matmul(
                    ps[:], VTi3[:, k, t * P:(t + 1) * P],
                    wv_all[:, k * FS:(k + 1) * FS],
                    start=(k == 0), stop=(k == DK - 1))
            if part == 1:
                v3 = vsb[t].rearrange("p (h w) -> p h w", w=VW)
                nc.vector.tensor_add(
                    v3[:, :, 0:HD],
                    ps.rearrange("p (h w) -> p h w", w=HD),
                    bvt.rearrange("p (h w) -> p h w", w=HD))
                nc.vector.memset(v3[:, :, HD:VW], 1.0)
        return g

    def o_granule(j, mt, oc, cn, ob_box, eng=None, use_sc=False):
        def g():
            if oc == 0:
                ob_box.append(outp.tile([P, 1024], bf16, tag="ob", name="ob"))
            ob = ob_box[-1]
            if use_sc:
                # tail only: scores are done, reuse the sc PSUM banks so
                # the outproj accumulators rotate over 4 banks
                big = sc_ps.tile([P, 1024], f32, tag="sc", name="oacc2")
                ps = big[:, 0:512]
            else:
                ps = acc_ps.tile([P, 512], f32, tag="acc", name="oacc")
            for f in range(2):
                nc.tensor.matmul(
                    ps[:], cn[f][:, mt * P:(mt + 1) * P],
                    wo_all[:, f * D + oc * 512: f * D + (oc + 1) * 512],
                    start=(f == 0), stop=(f == 1))
            dst = ob[:, oc * 512:(oc + 1) * 512]
            if eng == "scalar":
                nc.scalar.copy(dst, ps[:])
            elif eng == "gpsimd":
                nc.gpsimd.tensor_copy(dst, ps[:])
            else:
                nc.vector.tensor_copy(dst, ps[:])
            if oc == 1:
                nc.gpsimd.dma_start(
                    OUTP[j * 512 + mt * P: j * 512 + (mt + 1) * P, :], ob[:])
        return g

    # ---- filler bookkeeping ----
    # fillers[(sid, part)] = [deadline, earliest, fn].  Sequences of kind
    # k/q/v share one acc_ps accumulator across their two granules; at
    # most ONE such sequence may be open (part 0 popped, part 1 not).
    fillers = {}
    state = {"open": None}
    vsb_emitted = set()

    def add_seq(sid, earliest, deadline, fns):
        for part, fn in enumerate(fns):
            fillers[(sid, part)] = [deadline, earliest, fn]

    def _pop(sid, part):
        ent = fillers.pop((sid, part), None)
        if ent is None:
            return False
        ent[2]()
        if sid[0] in ("k", "q", "v"):
            state["open"] = sid if part == 0 else None
        if sid[0] == "v" and part == 1:
            vsb_emitted.add(sid[1])
        return True

    def close_open():
        if state["open"] is not None:
            _pop(state["open"], 1)

    def pop_seq_now(sid):
        if state["open"] is not None and state["open"] != sid:
            close_open()
        _pop(sid, 0)
        _pop(sid, 1)

    def scheduler_pop(astep, budget):
        # pop all past-deadline granules (free) + up to `budget` extras
        spent = 0
        while True:
            if state["open"] is not None:
                sid = state["open"]
                ent = fillers.get((sid, 1))
                due = ent is not None and ent[0] <= astep
                if not due and spent >= budget:
                    return
                _pop(sid, 1)
                if not due:
                    spent += 1
                continue
            best = None
            for (sid, part), ent in fillers.items():
                if part == 1 and (sid, 0) in fillers:
                    continue
                due = ent[0] <= astep
                if not due and ent[1] > astep:
                    continue
                key = (0 if due else 1, ent[0], ent[1], str(sid))
                if best is None or key < best[0]:
                    best = (key, sid, part, due)
            if best is None:
                return
            if not best[3] and spent >= budget:
                return
            _pop(best[1], best[2])
            if not best[3]:
                spent += 1

    # register filler sequences
    # kproj quarters: f is the pair index; scores (j=0,p,kt) need q=kt//4
    for f in range(2):
        for q in range(4):
            first_use = f * SK + 4 * q
            add_seq(("k", f, q), max(0, first_use - 8), first_use - 3,
                    [kq_granule(kT, wk_all, bkt, KTi3, f, q, p2, "k")
                     for p2 in range(2)])
    # qproj: qT[f] quarter j needed at astep (j*2+f)*SK
    for f in range(2):
        for j in range(NQ):
            first_use = (j * 2 + f) * SK
            add_seq(("q", f, j), max(0, first_use - 10), first_use - 4,
                    [kq_granule(qT, wq_all, bqt, QTi3, f, j, p2, "q")
                     for p2 in range(2)])
    # vproj: vsb[t] needed by ctx drain of (j=0, p=0, kt=t)
    for t in range(SK):
        add_seq(("v", t), max(0, t - 4), t,
                [v_granule(t, p2) for p2 in range(2)])

    # ================= attention =================
    pending = []           # (pair_serial, kt, ex, emit_astep)
    pair_cp = {}           # pair_serial -> [cp_even, cp_odd]
    pair_drained = {}
    ndrained = 0
    cn_byj = {}

    def drain_one():
        nonlocal ndrained
        ps_serial, kt, ex, _ = pending.pop(0)
        j, p = divmod(ps_serial, 2)
        if kt not in vsb_emitted:
            pop_seq_now(("v", kt))
        if ps_serial not in pair_cp:
            pair_cp[ps_serial] = [
                ctx_ps.tile([VW, 512], f32, tag="ctx", name=f"cp{ps_serial}h{i}")
                for i in range(2)]
            pair_drained[ps_serial] = 0
        cps = pair_cp[ps_serial]
        nd = pair_drained[ps_serial]
        for i in range(2):
            h = 2 * p + i
            nc.tensor.matmul(
                cps[i][:], vsb[kt][:, h * VW:(h + 1) * VW],
                ex[:, i * 512:(i + 1) * 512],
                start=(nd == 0), stop=(nd == SK - 1))
        pair_drained[ps_serial] = nd + 1
        ndrained += 1
        if nd == SK - 1:
            finish_pair(ps_serial)

    def finish_pair(ps_serial):
        j, p = divmod(ps_serial, 2)
        last = ps_serial == 2 * NQ - 1
        if j not in cn_byj:
            cn_byj[j] = [cnp.tile([P, 512], bf16, tag=f"cn{f}", name=f"cn{f}")
                         for f in range(2)]
        cn = cn_byj[j]
        cps = pair_cp.pop(ps_serial)
        for i in range(2):
            h = 2 * p + i
            if last:
                # tail: no PSUM pressure; skip staging, shortest chain
                src, srcsum = cps[i], cps[i][HD:HD + 1, :]
            else:
                cu = nrm.tile([VW, 512], f32, tag="cu", name="cu")
                with tc.high_priority(offset=5 * 10 ** 5):
                    nc.vector.tensor_copy(cu[:], cps[i][:])  # frees PSUM
                src, srcsum = cu, cu[HD:HD + 1, :]
            sm = nrm.tile([1, 512], f32, tag="sm", name="sm")
            nc.vector.tensor_copy(sm[:], srcsum)
            r1 = nrm.tile([1, 512], f32, tag="r1", name="r1")
            t1 = nrm.tile([1, 512], f32, tag="t1", name="t1")
            nc.vector.reciprocal_approx_accurate(r1[:], sm[:], t1[:])
            rb = nrm.tile([HD, 512], f32, tag="rb", name="rb")
            nc.gpsimd.partition_broadcast(rb[:], r1[:])
            nc.vector.tensor_mul(cn[p][(i) * HD:(i + 1) * HD, :],
                                 src[0:HD, :], rb[:])
        if p == 1:
            # both pairs of qchunk j normed -> register outproj fillers.
            # j>=2 outproj is reserved for the tail (keeps the PE warm
            # while the final norm chain runs).
            here = ps_serial * SK + SK - 1
            tail_j = j >= NQ - 2
            dl = 10 ** 9 if tail_j else (j + 1) * 2 * SK + 24
            early = 10 ** 9 if tail_j else here + 2
            for mt in range(4):
                ob_box = []
                eng = ("scalar", "vector") if tail_j else (None, None)
                add_seq(("o", j, mt), early, dl,
                        [o_granule(j, mt, oc, cn, ob_box, eng=eng[oc],
                                   use_sc=tail_j and (mt * 2 + oc) % 2 == 0)
                         for oc in (0, 1)])

    for j in range(NQ):
        for p in range(2):
            if j == 0:
                pop_seq_now(("k", p, 0))   # kproj first: K data lands first
            pop_seq_now(("q", p, j))
            for kt in range(SK):
                astep = (j * 2 + p) * SK + kt
                if kt % 4 == 0:
                    pop_seq_now(("k", p, kt // 4))
                # past-deadline fillers + pacing extras
                scheduler_pop(astep, 1)
                # drain ready ctx work (vsb emitted, exp old enough);
                # deeper lag at pair boundaries so the previous pair's
                # last ctx (waiting on its exp) cannot delay the new
                # pair's scores in the PE stream
                lag = 2 if kt < 2 else 1
                # hold a few ctx drains back during j0 so the
                # projection-heavy phase spills into j1's ACT slack
                thresh = max(0, 4 - max(0, astep - 31))
                while pending and pending[0][3] <= astep - lag and \
                        pending[0][1] in vsb_emitted and \
                        len(pending) > thresh:
                    drain_one()
                # backlog guard for the ex ring
                while len(pending) >= EXBUFS - 2:
                    drain_one()
                # scores (row-tiled pair) + exp; high priority keeps the
                # T0/T8 pair adjacent in the PE stream so they overlap
                sc = sc_ps.tile([P, 1024], f32, tag="sc", name="sc")
                with tc.high_priority(offset=10 ** 6):
                    for i in range(2):
                        nc.tensor.matmul(
                            sc[:, i * 512:(i + 1) * 512],
                            kT[p][i * HD:(i + 1) * HD, kt * P:(kt + 1) * P],
                            qT[p][i * HD:(i + 1) * HD, j * 512:(j + 1) * 512],
                            start=True, stop=True)
                ex = exq.tile([P, 1024], bf16, tag="ex", name="ex")
                nc.scalar.activation(ex[:], sc[:], EXP, scale=SCALE)
                pending.append(((j * 2 + p), kt, ex, astep))

    # tail: drain everything, then remaining fillers (outproj j=3)
    while pending:
        drain_one()
    close_open()
    while fillers:
        key = min(fillers.keys(),
                  key=lambda k: (fillers[k][0], str(k[0]), k[1]))
        _pop(*key)


_CACHE = {}


def _build():
    if "nc" in _CACHE:
        return _CACHE["nc"]
    nc = bacc.Bacc("TRN2", target_bir_lowering=False, debug=False)
    QTd = nc.dram_tensor("QT", [P, DK * S], bf16, kind="ExternalInput").ap()
    KTd = nc.dram_tensor("KT", [P, DK * S], bf16, kind="ExternalInput").ap()
    VTd = nc.dram_tensor("VT", [P, DK * S], bf16, kind="ExternalInput").ap()
    WqT = nc.dram_tensor("WqT", [P, DK * FS], bf16, kind="ExternalInput").ap()
    WkT = nc.dram_tensor("WkT", [P, DK * FS], bf16, kind="ExternalInput").ap()
    WvT = nc.dram_tensor("WvT", [P, DK * FS], bf16, kind="ExternalInput").ap()
    WoT = nc.dram_tensor("WoT", [P, 2 * D], bf16, kind="ExternalInput").ap()
    bq = nc.dram_tensor("bq", [P, 2], f32, kind="ExternalInput").ap()
    bk = nc.dram_tensor("bk", [P, 2], f32, kind="ExternalInput").ap()
    bv = nc.dram_tensor("bv", [1, FS], f32, kind="ExternalInput").ap()
    OUTP = nc.dram_tensor("OUTP", [S, D], bf16, kind="ExternalOutput").ap()
    with tile.TileContext(nc) as tc, ExitStack() as ctx:
        _emit(ctx, tc, nc,
              (QTd, KTd, VTd, WqT, WkT, WvT, WoT, bq, bk, bv, OUTP))
    nc.compile()
    _CACHE["nc"] = nc
    return nc


def _in_maps(Q, K, V, Wq, bq, Wk, bk, Wv, bv, Wo, bo):
    bf = ml_dtypes.bfloat16

    def packS(x):
        # x [S, D] activation -> xT [D, S] -> [p, q, k, c] quarter-major
        a = np.asarray(x, np.float32).T.astype(bf)          # [D, S]
        a = a.reshape(DK, P, NQ, 512).transpose(1, 2, 0, 3)
        return np.ascontiguousarray(a).reshape(P, DK * S)

    def packW(wT):
        # wT [D_in, F_out] -> [p, k, f]
        a = np.asarray(wT).astype(bf)
        k = a.shape[0] // P
        a = a.reshape(k, P, a.shape[1]).transpose(1, 0, 2)
        return np.ascontiguousarray(a).reshape(P, -1)

    QTb = [packS(Q[b]) for b in range(B)]
    KTb = [packS(K[b]) for b in range(B)]
    VTb = [packS(V[b]) for b in range(B)]
    c = np.ascontiguousarray
    maps = []
    for core in range(8):
        b, g = divmod(core, NG)
        sl = slice(g * FS, (g + 1) * FS)
        maps.append({
            "QT": QTb[b], "KT": KTb[b], "VT": VTb[b],
            "WqT": packW(np.asarray(Wq)[sl, :].T),
            "WkT": packW(np.asarray(Wk)[sl, :].T),
            "WvT": packW(np.asarray(Wv)[sl, :].T),
            "WoT": packW(np.asarray(Wo)[:, sl].T),
            "bq": c(np.asarray(bq, np.float32)[sl].reshape(2, P).T),
            "bk": c(np.asarray(bk, np.float32)[sl].reshape(2, P).T),
            "bv": c(np.asarray(bv, np.float32)[sl].reshape(1, FS)),
        })
    return maps


def kernel(Q, K, V, Wq, bq, Wk, bk, Wv, bv, Wo, bo):
    nc = _build()
    maps = _in_maps(Q, K, V, Wq, bq, Wk, bk, Wv, bv, Wo, bo)
    res = run_bass_kernel_spmd(nc, maps, core_ids=list(range(8)))
    out = np.empty((B, S, D), np.float32)
    for b in range(B):
        acc = res.results[b * NG]["OUTP"].astype(np.float32)
        for g in range(1, NG):
            acc = acc + res.results[b * NG + g]["OUTP"].astype(np.float32)
        out[b] = acc + np.asarray(bo, np.float32)[None, :]
    return out
